# revision 47
# baseline (speedup 1.0000x reference)
"""Dense MoE (all-experts, gate-weighted sum) on 8 Trainium2 NeuronCores.

Sharding: pure data-parallel over the token axis N (8192 -> 1024 rows/core);
every core holds all 8 experts, so no collectives are needed.

Mean-centered fp8 decomposition (the key trick):
    out = x @ Wbar.T  +  sum_e (g_e - 1/8) * (x @ W_e.T)  +  gates @ b_e
with Wbar = mean_e W_e. The bulk term x@Wbar runs in bf16; the 8 expert
GEMMs run as e4m3 DoubleRow matmuls (K=256/instruction -> 2x PE
throughput; measured 216 ns per FD=512 DR matmul, same as bf16). Because
the correction weights delta_e = g_e - 1/8 have std ~0.03, the fp8
quantization noise of the expert GEMMs is attenuated ~30x in the output.
The gating MLP's first layer also runs in fp8 DoubleRow (its dequant
scale is folded into b_g1 / W_g2 on the host, so zero extra device ops).
Measured end-to-end rel err ~1.5e-2 against the fp32 reference.

Phase order is chosen so the PE never waits on DMA:
  1. warmup fillers (HAM spin-up) while the first xq quads land
  2. gating DR GEMM, dk2-progressive as xq quads arrive; logits batched
     into one PSUM bank; single softmax; delta = (g-1/8)*DEQ on DVE;
     gates.T via 2 PE transposes (for the bias matmul later)
  3. the 8 fp8 experts stream back-to-back (64 DR MMs each); drains are
     one fused DVE op acc = (psum * delta) + acc, alternating psum banks
     across both pools
  4. the bf16 Wbar pass runs last: per tile one 9-MM PSUM group (8 dk
     matmuls + 1 bias matmul against a zero-padded per-strip b_e),
     drained by one fused DVE add that also converts to bf16 for output;
     output DMAs ride the gpsimd/scalar rings. Host upcasts bf16->fp32.

All bulk input traffic shares the sync ring, whose FIFO encodes priority:
xq quads, w0..w2, then x/Wbar (phase-4 inputs, transferred during the
wpool slack), then w3..w7 demand-paced. Sustained per-core DMA is only
~95 GB/s, so single-queue FIFO ordering beats spreading across queues
(parallel queues share the same aggregate bandwidth and just steal from
the expert-weight stream).
"""

import numpy as np
import ml_dtypes

import concourse.bass as bass
import concourse.mybir as mybir
import concourse.tile as tile
from concourse.bass_utils import run_bass_kernel_spmd

N, D, E, O, H = 8192, 1024, 8, 1024, 256
NCORES = 8
NLOC = N // NCORES          # 1024 rows per core
P = 128                     # partitions
NT = NLOC // P              # 8 n-tiles
DK = D // P                 # 8 contraction tiles (bf16 path)
DK2 = D // 256              # 4 double-row contraction tiles (fp8 path)
FO = 512                    # matmul moving free dim (one PSUM bank of fp32)
OH = O // FO                # 2 output halves
H2 = H // P                 # 2 h-tiles
BF16 = mybir.dt.bfloat16
F8E4 = mybir.dt.float8e4
F32 = mybir.dt.float32
BF = ml_dtypes.bfloat16
F8 = ml_dtypes.float8_e4m3fn
DR = mybir.MatmulPerfMode.DoubleRow
N_WARM = 14                 # pre-stream HAM warmup matmuls: ~6us of cold
                            # matmuls covers the worst-case first-xq-quad
                            # arrival, so the PE hits K=8/8 with no idle gap
                            # and the gating GEMM runs warm (216 vs 427 ns)
N_FILL = 2                  # fillers between gating and the expert stream
SX = 32.0                   # host scale for x -> e4m3
SW = 2048.0                 # host scale for W_e / W_g1 -> e4m3
DEQ = 1.0 / (SX * SW)       # folded into delta on device (experts) and
                            # into b_g1/W_g2 on host (gating)


def legalize_single_wait(nc, max_waits=1):
    """This walrus build rejects instructions carrying more than one sync
    wait. Split each multi-wait instruction: excess waits move onto fresh
    same-engine NoOps inserted immediately before it (identical semantics:
    the engine stalls at the same program point on every semaphore)."""
    for f in nc.m.functions:
        for blk in f.blocks:
            insts = list(blk.instructions)
            if all(
                (i.sync_info is None or len(i.sync_info.on_wait) <= max_waits)
                for i in insts
            ):
                continue
            new = []
            for inst in insts:
                si = inst.sync_info
                if si is not None and len(si.on_wait) > max_waits:
                    waits = list(si.on_wait)
                    for k, w in enumerate(waits[:-max_waits]):
                        nop = mybir.InstNoOp(name=f"{inst.name}-w{k}")
                        nop.engine = inst.engine
                        nop.sync_info = mybir.SyncInfo(on_wait=[w], on_update=[])
                        new.append(nop)
                    si.on_wait = waits[-max_waits:]
                new.append(inst)
            blk.instructions = new
    return nc


def build_moe():
    nc = bass.Bass(target_bir_lowering=False)
    xq = nc.dram_tensor("xq", [P, DK2, 2, NLOC], F8E4, kind="ExternalInput")
    xT = nc.dram_tensor("xT", [P, DK, NLOC], BF16, kind="ExternalInput")
    wbar = nc.dram_tensor("wbar", [P, DK, O], BF16, kind="ExternalInput")
    wq = nc.dram_tensor("wq", [E, P, DK2, 2, O], F8E4, kind="ExternalInput")
    wg1q = nc.dram_tensor("wg1q", [P, DK2, 2, H], F8E4, kind="ExternalInput")
    wg2t = nc.dram_tensor("wg2t", [P, H2, E], BF16, kind="ExternalInput")
    bg1 = nc.dram_tensor("bg1", [P, H2], F32, kind="ExternalInput")
    cexp = nc.dram_tensor("cexp", [P, 4, E], F32, kind="ExternalInput")
    be_c = nc.dram_tensor("be_c", [E, O], BF16, kind="ExternalInput")
    ident = nc.dram_tensor("ident", [P, P], F32, kind="ExternalInput")
    out = nc.dram_tensor("out", [NT, OH, P, FO], BF16, kind="ExternalOutput")

    with tile.TileContext(nc) as tc:
        with (
            tc.tile_pool(name="const", bufs=1) as constp,
            tc.tile_pool(name="wpool", bufs=3) as wpool,
            tc.tile_pool(name="work", bufs=4) as workp,
            tc.tile_pool(name="g_ps", bufs=4, space="PSUM") as gp,
            tc.tile_pool(name="mm_ps", bufs=4, space="PSUM") as mmp,
        ):
            # ---- PE warm-up + ACT Exp-table preload during the DMA wait ----
            warm_a = constp.tile([P, P], BF16, tag="warm_a")
            nc.gpsimd.memset(warm_a, 0.0)
            warm_b = constp.tile([P, FO], BF16, tag="warm_b")
            nc.gpsimd.memset(warm_b, 0.0)
            for i in range(N_WARM):
                wpsum = mmp.tile([P, FO], F32, tag="mm", name=f"warm{i}")
                nc.tensor.matmul(wpsum, warm_a, warm_b, start=True, stop=True)
            dummy_exp = workp.tile([1, 1], F32, tag="dummy")
            nc.scalar.activation(
                out=dummy_exp,
                in_=warm_b[0:1, 0:1],
                func=mybir.ActivationFunctionType.Exp,
            )

            # ---- resident inputs.
            # sync ring (FIFO = priority): xq quads, then fp8 experts.
            # scalar ring: x bf16 + Wbar (needed only for the tail phase).
            # gpsimd ring: gating weights + small consts + bias ----
            xq_sb = constp.tile([P, DK2, 2, NLOC], F8E4, tag="xq")
            wg1q_sb = constp.tile([P, DK2, 2, H], F8E4, tag="wg1q")
            nc.gpsimd.dma_start(out=wg1q_sb, in_=wg1q[:, :, :, :])
            nc.sync.dma_start(out=xq_sb[:, 0:1, :, :], in_=xq[:, 0:1, :, :])
            nc.sync.dma_start(out=xq_sb[:, 1:2, :, :], in_=xq[:, 1:2, :, :])
            nc.sync.dma_start(out=xq_sb[:, 2:4, :, :], in_=xq[:, 2:4, :, :])

            wg2t_sb = constp.tile([P, H2, E], BF16, tag="wg2t")
            nc.gpsimd.dma_start(out=wg2t_sb, in_=wg2t[:, :, :])
            bg1_sb = constp.tile([P, H2], F32, tag="bg1")
            nc.gpsimd.dma_start(out=bg1_sb, in_=bg1[:, :])
            cexp_sb = constp.tile([P, 4, E], F32, tag="cexp")
            nc.gpsimd.dma_start(out=cexp_sb, in_=cexp[:, :, :])
            ident_sb = constp.tile([P, P], F32, tag="ident")
            nc.gpsimd.dma_start(out=ident_sb, in_=ident[:, :])
            # zero-padded per-strip bias assembled on device from a 16KB
            # compact transfer (saves ~1MB of HBM traffic)
            be_small = constp.tile([E, O], BF16, tag="be_c")
            nc.gpsimd.dma_start(out=be_small, in_=be_c[:, :])
            be_sb = constp.tile([P, 4, O], BF16, tag="be_rep")
            nc.gpsimd.memset(be_sb, 0.0)
            for a in range(4):
                nc.gpsimd.dma_start(
                    out=be_sb[32 * a : 32 * a + E, a, :], in_=be_small
                )

            xT_sb = constp.tile([P, DK, NLOC], BF16, tag="xT")
            wbar_sb = constp.tile([P, DK, O], BF16, tag="wbar")

            # ---- gating DR GEMM (4 psum banks), dk2-progressive ----
            hT_sb = [
                constp.tile([P, NLOC], BF16, tag=f"hT{h2}", name=f"hT{h2}")
                for h2 in range(H2)
            ]
            psum_g = {
                (h2, nh): gp.tile([P, FO], F32, tag="g", name=f"psum_g{h2}_{nh}")
                for h2 in range(H2)
                for nh in range(NLOC // FO)
            }

            def gating_mms(dk2s):
                for dk2 in dk2s:
                    for h2 in range(H2):
                        for nh in range(NLOC // FO):
                            nc.tensor.matmul(
                                psum_g[(h2, nh)],
                                wg1q_sb[:, dk2, :, h2 * P : (h2 + 1) * P],
                                xq_sb[:, dk2, :, nh * FO : (nh + 1) * FO],
                                start=(dk2 == 0),
                                stop=(dk2 == DK2 - 1),
                                perf_mode=DR,
                            )

            for dk2 in range(DK2):
                gating_mms([dk2])

            # relus on DVE: hT' = max(psum + bg1*SX*SW, 0) stored in bf16
            # (scaled h; the 1/(SX*SW) is folded into wg2t on host)
            for nh in range(NLOC // FO):
                for h2 in range(H2):
                    nc.vector.tensor_scalar(
                        out=hT_sb[h2][:, nh * FO : (nh + 1) * FO],
                        in0=psum_g[(h2, nh)],
                        scalar1=bg1_sb[:, h2 : h2 + 1],
                        scalar2=0.0,
                        op0=mybir.AluOpType.add,
                        op1=mybir.AluOpType.max,
                    )

            # logits for all n-tiles in ONE psum bank (recycled g pool);
            # b_g2 is applied post-exp as a multiplicative exp(b_g2) fold,
            # saving a K=1 bias matmul per n-tile
            psum_l = gp.tile([P, NT, E], F32, tag="g", name="psum_l")
            for nt in range(NT):
                for h2 in range(H2):
                    nc.tensor.matmul(
                        psum_l[:, nt, :],
                        hT_sb[h2][:, nt * P : (nt + 1) * P],
                        wg2t_sb[:, h2, :],
                        start=(h2 == 0),
                        stop=(h2 == H2 - 1),
                    )

            # batched softmax (no max-subtract: logits are O(1)); gates
            # zero-padded to 32 per n-tile so the transposed layout is
            # 32-row aligned
            EP = 32
            gates_g = [
                constp.tile([P, NT // 2, EP], F32, tag=f"gates{g}", name=f"gates{g}")
                for g in range(2)
            ]
            for g in range(2):
                nc.vector.memset(gates_g[g], 0.0)
            gates_at = lambda nt: gates_g[nt // 4][:, nt % 4, 0:E]
            for g in range(2):
                nc.scalar.activation(
                    out=gates_g[g][:, :, 0:E],
                    in_=psum_l[:, 4 * g : 4 * (g + 1), :],
                    func=mybir.ActivationFunctionType.Exp,
                )
            for g in range(2):
                nc.vector.tensor_mul(
                    gates_g[g][:, :, 0:E], gates_g[g][:, :, 0:E], cexp_sb
                )
            sumexp = workp.tile([P, NT, 1], F32, tag="sumexp")
            for g in range(2):
                nc.vector.reduce_sum(
                    sumexp[:, 4 * g : 4 * (g + 1), :],
                    gates_g[g][:, :, 0:E],
                    axis=mybir.AxisListType.X,
                )
            rsum = workp.tile([P, NT, 1], F32, tag="rsum")
            nc.vector.reciprocal(rsum, sumexp)
            for nt in range(NT):
                nc.vector.tensor_scalar_mul(
                    gates_at(nt), gates_at(nt), rsum[:, nt, :]
                )

            # delta = (gates - 1/8) * DEQ  (dequant scale folded in);
            # padding columns get garbage but are never read
            delta_g = [
                constp.tile([P, NT // 2, EP], F32, tag=f"delta{g}", name=f"delta{g}")
                for g in range(2)
            ]
            for g in range(2):
                nc.vector.tensor_scalar(
                    out=delta_g[g],
                    in0=gates_g[g],
                    scalar1=-1.0 / E,
                    scalar2=DEQ,
                    op0=mybir.AluOpType.add,
                    op1=mybir.AluOpType.mult,
                )
            delta_at = lambda nt: delta_g[nt // 4][:, nt % 4, 0:E]

            # fillers bridging any gap until the first expert's weights land
            for i in range(N_FILL):
                wpsum = mmp.tile([P, FO], F32, tag="mm", name=f"fill{i}")
                nc.tensor.matmul(wpsum, warm_a, warm_b, start=True, stop=True)

            # gates.T via two 128x128 PE transposes (rows nt*32+e,
            # 32-aligned) for the bias matmuls in the tail phase
            gatesT = []
            for g in range(2):
                psum_t = gp.tile([P, P], F32, tag="g", name="psum_t")
                nc.tensor.transpose(
                    psum_t, gates_g[g].rearrange("p a b -> p (a b)"), ident_sb
                )
                gT = constp.tile([P, P], BF16, tag=f"gatesT{g}", name=f"gatesT{g}")
                nc.scalar.copy(out=gT, in_=psum_t)
                gatesT.append(gT)

            acc_sb = [
                [
                    constp.tile(
                        [P, FO], F32, tag=f"acc{nt}_{oh}", name=f"acc{nt}_{oh}"
                    )
                    for oh in range(OH)
                ]
                for nt in range(NT)
            ]
            # bf16 staging tiles for the final output
            out_sb = [
                [
                    constp.tile(
                        [P, FO], BF16, tag=f"out{nt}_{oh}", name=f"out{nt}_{oh}"
                    )
                    for oh in range(OH)
                ]
                for nt in range(NT)
            ]

            # ---- phase 3: stream fp8 experts, accumulate delta-weighted
            # DoubleRow GEMMs. Drains: acc = (psum * delta) + acc ----
            for e in range(E):
                w_all = wpool.tile([P, DK2, 2, O], F8E4, tag="w", name=f"w{e}")
                nc.sync.dma_start(out=w_all, in_=wq[e])
                if e == 2:
                    # x/Wbar (phase-4 inputs) ride the sync ring behind the
                    # first three expert weights: the single queue keeps the
                    # full ~93 GB/s and its FIFO gives the head xq/w0
                    # transfers strict priority; these 4MB then transfer
                    # during the wpool slack and land ~90us before phase 4
                    nc.sync.dma_start(out=xT_sb, in_=xT[:, :, :])
                    nc.sync.dma_start(out=wbar_sb, in_=wbar[:, :, :])
                for oh in range(OH):
                    for nt in range(NT):
                        dcol = delta_at(nt)[:, e : e + 1]
                        acc = acc_sb[nt][oh]
                        # alternate psum between both pools -> ~8 banks in
                        # flight, drains never gate the PE
                        if (oh * NT + nt) % 2 == 0:
                            psum = mmp.tile([P, FO], F32, tag="mm")
                        else:
                            psum = gp.tile([P, FO], F32, tag="g", name="psum_e")
                        for dk in range(DK2):
                            nc.tensor.matmul(
                                psum,
                                xq_sb[:, dk, :, nt * P : (nt + 1) * P],
                                w_all[:, dk, :, oh * FO : (oh + 1) * FO],
                                start=(dk == 0),
                                stop=(dk == DK2 - 1),
                                perf_mode=DR,
                            )
                        if e == 0:
                            # first expert initializes acc; split ACT/DVE
                            if (oh * NT + nt) % 2 == 0:
                                nc.scalar.mul(acc, psum, dcol)
                            else:
                                nc.vector.tensor_scalar_mul(acc, psum, dcol)
                        else:
                            nc.vector.scalar_tensor_tensor(
                                out=acc,
                                in0=psum,
                                scalar=dcol,
                                in1=acc,
                                op0=mybir.AluOpType.mult,
                                op1=mybir.AluOpType.add,
                            )

            # ---- phase 4: bf16 Wbar pass + bias, fused final drains.
            # Per tile one 9-matmul PSUM group: 8 dk matmuls of x@Wbar plus
            # one bias matmul (gatesT[g] @ zero-padded per-strip b_e); the
            # drain adds acc and converts to bf16, then DMAs out ----
            for j, (nt, oh) in enumerate(
                [(nt, oh) for oh in range(OH) for nt in range(NT)]
            ):
                g, a = nt // 4, nt % 4
                pool, tg = ((mmp, "mm"), (gp, "g"))[j % 2]
                psum = pool.tile([P, FO], F32, tag=tg, name=f"wb{nt}_{oh}")
                for dk in range(DK):
                    nc.tensor.matmul(
                        psum,
                        xT_sb[:, dk, nt * P : (nt + 1) * P],
                        wbar_sb[:, dk, oh * FO : (oh + 1) * FO],
                        start=(dk == 0),
                        stop=False,
                    )
                nc.tensor.matmul(
                    psum,
                    gatesT[g],
                    be_sb[:, a, oh * FO : (oh + 1) * FO],
                    start=False,
                    stop=True,
                )
                if j == 2 * NT - 1:
                    # final tile: drain + DMA in 256-wide chunks so the DMA
                    # of the first half overlaps the drain of the second
                    for hh in range(2):
                        sl = slice(hh * 256, (hh + 1) * 256)
                        nc.vector.tensor_add(
                            out_sb[nt][oh][:, sl], acc_sb[nt][oh][:, sl],
                            psum[:, sl],
                        )
                        oslice = out[nt, oh, :, sl]
                        if hh == 0:
                            nc.sync.dma_start(out=oslice, in_=out_sb[nt][oh][:, sl])
                        else:
                            nc.scalar.dma_start(out=oslice, in_=out_sb[nt][oh][:, sl])
                else:
                    nc.vector.tensor_add(
                        out_sb[nt][oh], acc_sb[nt][oh], psum
                    )
                    oslice = out[nt, oh, :, :]
                    # outputs stay off the sync ring (which must drain the
                    # expert-weight stream) and off the scalar ring's
                    # x/Wbar transfers: alternate gpsimd/scalar
                    if j % 2 == 0:
                        nc.gpsimd.dma_start(out=oslice, in_=out_sb[nt][oh])
                    else:
                        nc.scalar.dma_start(out=oslice, in_=out_sb[nt][oh])

    legalize_single_wait(nc)
    return nc


_NC_CACHE = {}


def _get_nc():
    if "nc" not in _NC_CACHE:
        _NC_CACHE["nc"] = build_moe()
    return _NC_CACHE["nc"]


def make_in_maps(x, W_e, b_e, W_g1, b_g1, W_g2, b_g2):
    x = np.asarray(x, dtype=np.float32)
    W_e = np.asarray(W_e, dtype=np.float32)
    # fp8 expert weights, mean-centered (sum_e delta_e = 0 makes this
    # equivalent, and the smaller elements carry ~6% less fp8 noise):
    # [E, O, D] -> [E, P, DK2, 2, O] with logical d = dk2*256 + i*128 + p
    wbar_f = W_e.mean(axis=0)             # [O, D]
    wq = (
        ((W_e - wbar_f) * SW)
        .astype(F8)
        .transpose(0, 2, 1)               # [E, D, O]
        .reshape(E, DK2, 2, P, O)
        .transpose(0, 3, 1, 2, 4)         # [E, P, DK2, 2, O]
    )
    wq = np.ascontiguousarray(wq)
    # bf16 mean expert: [P, DK, O]
    wbar = wbar_f.T.reshape(DK, P, O).transpose(1, 0, 2)
    wbar = np.ascontiguousarray(wbar).astype(BF)
    # fp8 gating layer-1 weights, same d-pairing as xq
    wg1q = (
        (np.asarray(W_g1, dtype=np.float32) * SW)
        .astype(F8)
        .T.reshape(DK2, 2, P, H)
        .transpose(2, 0, 1, 3)            # [P, DK2, 2, H]
    )
    wg1q = np.ascontiguousarray(wg1q)
    # wg2 absorbs the gating dequant 1/(SX*SW); bg1 is pre-scaled by SX*SW
    wg2t = (
        (np.asarray(W_g2, dtype=np.float32) * DEQ)
        .T.reshape(H2, P, E)
        .transpose(1, 0, 2)
    )
    wg2t = np.ascontiguousarray(wg2t).astype(BF)
    bg1 = np.ascontiguousarray(
        (np.asarray(b_g1, dtype=np.float32) * (SX * SW)).reshape(H2, P).T
    )
    cexp_np = np.ascontiguousarray(
        np.broadcast_to(
            np.exp(np.asarray(b_g2, dtype=np.float32))[None, None, :], (P, 4, E)
        )
    ).astype(np.float32)
    be_c = np.ascontiguousarray(np.asarray(b_e, dtype=np.float32)).astype(BF)
    ident_np = np.eye(P, dtype=np.float32)
    xb = x.astype(BF)
    xq8 = (x * SX).astype(F8)
    in_maps = []
    for c in range(NCORES):
        xrow = xb[c * NLOC : (c + 1) * NLOC, :]
        xT_c = np.asarray(xrow.T).reshape(DK, P, NLOC).transpose(1, 0, 2)
        xT_c = np.ascontiguousarray(xT_c)
        xq_c = (
            np.asarray(xq8[c * NLOC : (c + 1) * NLOC, :].T)
            .reshape(DK2, 2, P, NLOC)
            .transpose(2, 0, 1, 3)        # [P, DK2, 2, NLOC]
        )
        xq_c = np.ascontiguousarray(xq_c)
        in_maps.append(
            {
                "xT": xT_c,
                "xq": xq_c,
                "wbar": wbar,
                "wq": wq,
                "wg1q": wg1q,
                "wg2t": wg2t,
                "bg1": bg1,
                "cexp": cexp_np,
                "be_c": be_c,
                "ident": ident_np,
            }
        )
    return in_maps


def kernel(x, W_e, b_e, W_g1, b_g1, W_g2, b_g2, **run_kwargs):
    nc = _get_nc()
    in_maps = make_in_maps(x, W_e, b_e, W_g1, b_g1, W_g2, b_g2)
    res = run_bass_kernel_spmd(nc, in_maps, core_ids=list(range(NCORES)), **run_kwargs)
    outs = []
    for c in range(NCORES):
        o = np.asarray(res.results[c]["out"])  # [NT, OH, P, FO] bf16
        outs.append(
            o.astype(np.float32).transpose(0, 2, 1, 3).reshape(NLOC, O)
        )
    out = np.concatenate(outs, axis=0)
    if run_kwargs:
        kernel.last_results = res
    return out


if __name__ == "__main__":
    rng = np.random.default_rng(0)
    s = 1.0 / np.sqrt(D)
    sh = 1.0 / np.sqrt(H)
    inputs = {
        "x": rng.standard_normal((N, D), dtype=np.float32),
        "W_e": rng.uniform(-s, s, (E, O, D)).astype(np.float32),
        "b_e": rng.uniform(-s, s, (E, O)).astype(np.float32),
        "W_g1": rng.uniform(-s, s, (H, D)).astype(np.float32),
        "b_g1": rng.uniform(-sh, sh, (H,)).astype(np.float32),
        "W_g2": rng.uniform(-sh, sh, (E, H)).astype(np.float32),
        "b_g2": rng.uniform(-sh, sh, (E,)).astype(np.float32),
    }
    out = kernel(**inputs)
    print("out", out.shape, out.dtype, float(np.abs(out).max()))


# revision 49
# speedup vs baseline: 1.1871x; 1.1871x over previous
"""Dense MoE (all-experts, gate-weighted sum) on 8 Trainium2 NeuronCores.

Sharding: pure data-parallel over the token axis N (8192 -> 1024 rows/core);
every core holds all 8 experts, so no collectives are needed.

Mean-centered fp8 decomposition (the key trick):
    out = x @ Wbar.T  +  sum_e (g_e - 1/8) * (x @ W_e.T)  +  gates @ b_e
with Wbar = mean_e W_e. The bulk term x@Wbar runs in bf16; the 8 expert
GEMMs run as e4m3 DoubleRow matmuls (K=256/instruction -> 2x PE
throughput; measured 216 ns per FD=512 DR matmul, same as bf16). Because
the correction weights delta_e = g_e - 1/8 have std ~0.03, the fp8
quantization noise of the expert GEMMs is attenuated ~30x in the output.
The gating MLP's first layer also runs in fp8 DoubleRow (its dequant
scale is folded into b_g1 / W_g2 on the host, so zero extra device ops).
Measured end-to-end rel err ~1.5e-2 against the fp32 reference.

Phase order is chosen so the PE never waits on DMA:
  1. warmup fillers (HAM spin-up) while the first xq quads land
  2. gating DR GEMM, dk2-progressive as xq quads arrive; logits batched
     into one PSUM bank; single softmax; delta = (g-1/8)*DEQ on DVE;
     gates.T via 2 PE transposes (for the bias matmul later)
  3. the 8 fp8 experts stream back-to-back (64 DR MMs each); drains are
     one fused DVE op acc = (psum * delta) + acc, alternating psum banks
     across both pools
  4. the bf16 Wbar pass runs last: per tile one 9-MM PSUM group (8 dk
     matmuls + 1 bias matmul against a zero-padded per-strip b_e),
     drained by one fused DVE add that also converts to bf16 for output;
     output DMAs ride the gpsimd/scalar rings. Host upcasts bf16->fp32.

All bulk input traffic shares the sync ring, whose FIFO encodes priority:
xq quads, w0..w2, then x/Wbar (phase-4 inputs, transferred during the
wpool slack), then w3..w7 demand-paced. Sustained per-core DMA is only
~95 GB/s, so single-queue FIFO ordering beats spreading across queues
(parallel queues share the same aggregate bandwidth and just steal from
the expert-weight stream).
"""

import numpy as np
import ml_dtypes

import concourse.bass as bass
import concourse.mybir as mybir
import concourse.tile as tile
from concourse.bass_utils import run_bass_kernel_spmd

N, D, E, O, H = 8192, 1024, 8, 1024, 256
NCORES = 8
NLOC = N // NCORES          # 1024 rows per core
P = 128                     # partitions
NT = NLOC // P              # 8 n-tiles
DK = D // P                 # 8 contraction tiles (bf16 path)
DK2 = D // 256              # 4 double-row contraction tiles (fp8 path)
FO = 512                    # matmul moving free dim (one PSUM bank of fp32)
OH = O // FO                # 2 output halves
H2 = H // P                 # 2 h-tiles
BF16 = mybir.dt.bfloat16
F8E4 = mybir.dt.float8e4
F32 = mybir.dt.float32
BF = ml_dtypes.bfloat16
F8 = ml_dtypes.float8_e4m3fn
DR = mybir.MatmulPerfMode.DoubleRow
N_WARM = 14                 # pre-stream HAM warmup matmuls: ~6us of cold
                            # matmuls covers the worst-case first-xq-quad
                            # arrival, so the PE hits K=8/8 with no idle gap
                            # and the gating GEMM runs warm (216 vs 427 ns)
N_FILL = 2                  # fillers between gating and the expert stream
SX = 32.0                   # host scale for x -> e4m3
SW = 2048.0                 # host scale for W_e / W_g1 -> e4m3
DEQ = 1.0 / (SX * SW)       # folded into delta on device (experts) and
                            # into b_g1/W_g2 on host (gating)


def legalize_single_wait(nc, max_waits=1):
    """This walrus build rejects instructions carrying more than one sync
    wait. Split each multi-wait instruction: excess waits move onto fresh
    same-engine NoOps inserted immediately before it (identical semantics:
    the engine stalls at the same program point on every semaphore)."""
    for f in nc.m.functions:
        for blk in f.blocks:
            insts = list(blk.instructions)
            if all(
                (i.sync_info is None or len(i.sync_info.on_wait) <= max_waits)
                for i in insts
            ):
                continue
            new = []
            for inst in insts:
                si = inst.sync_info
                if si is not None and len(si.on_wait) > max_waits:
                    waits = list(si.on_wait)
                    for k, w in enumerate(waits[:-max_waits]):
                        nop = mybir.InstNoOp(name=f"{inst.name}-w{k}")
                        nop.engine = inst.engine
                        nop.sync_info = mybir.SyncInfo(on_wait=[w], on_update=[])
                        new.append(nop)
                    si.on_wait = waits[-max_waits:]
                new.append(inst)
            blk.instructions = new
    return nc


def build_moe():
    nc = bass.Bass(target_bir_lowering=False)
    xq = nc.dram_tensor("xq", [P, DK2, 2, NLOC], F8E4, kind="ExternalInput")
    xT = nc.dram_tensor("xT", [P, DK, NLOC], BF16, kind="ExternalInput")
    wbar = nc.dram_tensor("wbar", [P, DK, O], BF16, kind="ExternalInput")
    wq = nc.dram_tensor("wq", [E, P, DK2, 2, O], F8E4, kind="ExternalInput")
    wg1q = nc.dram_tensor("wg1q", [P, DK2, 2, H], F8E4, kind="ExternalInput")
    wg2t = nc.dram_tensor("wg2t", [P, H2, E], BF16, kind="ExternalInput")
    bg1 = nc.dram_tensor("bg1", [P, H2], F32, kind="ExternalInput")
    cexp = nc.dram_tensor("cexp", [P, 4, E], F32, kind="ExternalInput")
    be_c = nc.dram_tensor("be_c", [E, O], BF16, kind="ExternalInput")
    ident = nc.dram_tensor("ident", [P, P], F32, kind="ExternalInput")
    out = nc.dram_tensor("out", [NT, OH, P, FO], BF16, kind="ExternalOutput")

    with tile.TileContext(nc) as tc:
        with (
            tc.tile_pool(name="const", bufs=1) as constp,
            tc.tile_pool(name="wpool", bufs=3) as wpool,
            tc.tile_pool(name="work", bufs=4) as workp,
            tc.tile_pool(name="g_ps", bufs=4, space="PSUM") as gp,
            tc.tile_pool(name="mm_ps", bufs=4, space="PSUM") as mmp,
        ):
            # ---- PE warm-up + ACT Exp-table preload during the DMA wait ----
            warm_a = constp.tile([P, P], BF16, tag="warm_a")
            nc.gpsimd.memset(warm_a, 0.0)
            warm_b = constp.tile([P, FO], BF16, tag="warm_b")
            nc.gpsimd.memset(warm_b, 0.0)
            for i in range(N_WARM):
                wpsum = mmp.tile([P, FO], F32, tag="mm", name=f"warm{i}")
                nc.tensor.matmul(wpsum, warm_a, warm_b, start=True, stop=True)
            dummy_exp = workp.tile([1, 1], F32, tag="dummy")
            nc.scalar.activation(
                out=dummy_exp,
                in_=warm_b[0:1, 0:1],
                func=mybir.ActivationFunctionType.Exp,
            )

            # ---- resident inputs.
            # sync ring (FIFO = priority): xq quads, then fp8 experts.
            # scalar ring: x bf16 + Wbar (needed only for the tail phase).
            # gpsimd ring: gating weights + small consts + bias ----
            xq_sb = constp.tile([P, DK2, 2, NLOC], F8E4, tag="xq")
            wg1q_sb = constp.tile([P, DK2, 2, H], F8E4, tag="wg1q")
            nc.gpsimd.dma_start(out=wg1q_sb, in_=wg1q[:, :, :, :])
            nc.sync.dma_start(out=xq_sb[:, 0:1, :, :], in_=xq[:, 0:1, :, :])
            nc.sync.dma_start(out=xq_sb[:, 1:2, :, :], in_=xq[:, 1:2, :, :])
            nc.sync.dma_start(out=xq_sb[:, 2:4, :, :], in_=xq[:, 2:4, :, :])

            wg2t_sb = constp.tile([P, H2, E], BF16, tag="wg2t")
            nc.gpsimd.dma_start(out=wg2t_sb, in_=wg2t[:, :, :])
            bg1_sb = constp.tile([P, H2], F32, tag="bg1")
            nc.gpsimd.dma_start(out=bg1_sb, in_=bg1[:, :])
            cexp_sb = constp.tile([P, 4, E], F32, tag="cexp")
            nc.gpsimd.dma_start(out=cexp_sb, in_=cexp[:, :, :])
            ident_sb = constp.tile([P, P], F32, tag="ident")
            nc.gpsimd.dma_start(out=ident_sb, in_=ident[:, :])
            # zero-padded per-strip bias assembled on device from a 16KB
            # compact transfer (saves ~1MB of HBM traffic)
            be_small = constp.tile([E, O], BF16, tag="be_c")
            nc.gpsimd.dma_start(out=be_small, in_=be_c[:, :])
            be_sb = constp.tile([P, 4, O], BF16, tag="be_rep")
            nc.gpsimd.memset(be_sb, 0.0)
            for a in range(4):
                nc.gpsimd.dma_start(
                    out=be_sb[32 * a : 32 * a + E, a, :], in_=be_small
                )

            xT_sb = constp.tile([P, DK, NLOC], BF16, tag="xT")
            wbar_sb = constp.tile([P, DK, O], BF16, tag="wbar")

            # ---- gating DR GEMM (4 psum banks), dk2-progressive ----
            hT_sb = [
                constp.tile([P, NLOC], BF16, tag=f"hT{h2}", name=f"hT{h2}")
                for h2 in range(H2)
            ]
            psum_g = {
                (h2, nh): gp.tile([P, FO], F32, tag="g", name=f"psum_g{h2}_{nh}")
                for h2 in range(H2)
                for nh in range(NLOC // FO)
            }

            def gating_mms(dk2s):
                for dk2 in dk2s:
                    for h2 in range(H2):
                        for nh in range(NLOC // FO):
                            nc.tensor.matmul(
                                psum_g[(h2, nh)],
                                wg1q_sb[:, dk2, :, h2 * P : (h2 + 1) * P],
                                xq_sb[:, dk2, :, nh * FO : (nh + 1) * FO],
                                start=(dk2 == 0),
                                stop=(dk2 == DK2 - 1),
                                perf_mode=DR,
                            )

            for dk2 in range(DK2):
                gating_mms([dk2])

            # relus on DVE: hT' = max(psum + bg1*SX*SW, 0) stored in bf16
            # (scaled h; the 1/(SX*SW) is folded into wg2t on host)
            for nh in range(NLOC // FO):
                for h2 in range(H2):
                    nc.vector.tensor_scalar(
                        out=hT_sb[h2][:, nh * FO : (nh + 1) * FO],
                        in0=psum_g[(h2, nh)],
                        scalar1=bg1_sb[:, h2 : h2 + 1],
                        scalar2=0.0,
                        op0=mybir.AluOpType.add,
                        op1=mybir.AluOpType.max,
                    )

            # logits for all n-tiles in ONE psum bank (recycled g pool);
            # b_g2 is applied post-exp as a multiplicative exp(b_g2) fold,
            # saving a K=1 bias matmul per n-tile
            psum_l = gp.tile([P, NT, E], F32, tag="g", name="psum_l")
            for nt in range(NT):
                for h2 in range(H2):
                    nc.tensor.matmul(
                        psum_l[:, nt, :],
                        hT_sb[h2][:, nt * P : (nt + 1) * P],
                        wg2t_sb[:, h2, :],
                        start=(h2 == 0),
                        stop=(h2 == H2 - 1),
                    )

            # batched softmax (no max-subtract: logits are O(1)); gates
            # zero-padded to 32 per n-tile so the transposed layout is
            # 32-row aligned
            EP = 32
            gates_g = [
                constp.tile([P, NT // 2, EP], F32, tag=f"gates{g}", name=f"gates{g}")
                for g in range(2)
            ]
            for g in range(2):
                nc.vector.memset(gates_g[g], 0.0)
            gates_at = lambda nt: gates_g[nt // 4][:, nt % 4, 0:E]
            for g in range(2):
                nc.scalar.activation(
                    out=gates_g[g][:, :, 0:E],
                    in_=psum_l[:, 4 * g : 4 * (g + 1), :],
                    func=mybir.ActivationFunctionType.Exp,
                )
            for g in range(2):
                nc.vector.tensor_mul(
                    gates_g[g][:, :, 0:E], gates_g[g][:, :, 0:E], cexp_sb
                )
            sumexp = workp.tile([P, NT, 1], F32, tag="sumexp")
            for g in range(2):
                nc.vector.reduce_sum(
                    sumexp[:, 4 * g : 4 * (g + 1), :],
                    gates_g[g][:, :, 0:E],
                    axis=mybir.AxisListType.X,
                )
            rsum = workp.tile([P, NT, 1], F32, tag="rsum")
            nc.vector.reciprocal(rsum, sumexp)
            for nt in range(NT):
                nc.vector.tensor_scalar_mul(
                    gates_at(nt), gates_at(nt), rsum[:, nt, :]
                )

            # delta = (gates - 1/8) * DEQ  (dequant scale folded in);
            # padding columns get garbage but are never read
            delta_g = [
                constp.tile([P, NT // 2, EP], F32, tag=f"delta{g}", name=f"delta{g}")
                for g in range(2)
            ]
            for g in range(2):
                nc.vector.tensor_scalar(
                    out=delta_g[g],
                    in0=gates_g[g],
                    scalar1=-1.0 / E,
                    scalar2=DEQ,
                    op0=mybir.AluOpType.add,
                    op1=mybir.AluOpType.mult,
                )
            delta_at = lambda nt: delta_g[nt // 4][:, nt % 4, 0:E]

            # fillers bridging any gap until the first expert's weights land
            for i in range(N_FILL):
                wpsum = mmp.tile([P, FO], F32, tag="mm", name=f"fill{i}")
                nc.tensor.matmul(wpsum, warm_a, warm_b, start=True, stop=True)

            # gates.T via two 128x128 PE transposes (rows nt*32+e,
            # 32-aligned) for the bias matmuls in the tail phase
            gatesT = []
            for g in range(2):
                psum_t = gp.tile([P, P], F32, tag="g", name="psum_t")
                nc.tensor.transpose(
                    psum_t, gates_g[g].rearrange("p a b -> p (a b)"), ident_sb
                )
                gT = constp.tile([P, P], BF16, tag=f"gatesT{g}", name=f"gatesT{g}")
                nc.scalar.copy(out=gT, in_=psum_t)
                gatesT.append(gT)

            acc_sb = [
                [
                    constp.tile(
                        [P, FO], F32, tag=f"acc{nt}_{oh}", name=f"acc{nt}_{oh}"
                    )
                    for oh in range(OH)
                ]
                for nt in range(NT)
            ]
            # bf16 staging tiles for the final output
            out_sb = [
                [
                    constp.tile(
                        [P, FO], BF16, tag=f"out{nt}_{oh}", name=f"out{nt}_{oh}"
                    )
                    for oh in range(OH)
                ]
                for nt in range(NT)
            ]

            # ---- phase 3: stream fp8 experts, accumulate delta-weighted
            # DoubleRow GEMMs. Drains: acc = (psum * delta) + acc ----
            for e in range(E):
                w_all = wpool.tile([P, DK2, 2, O], F8E4, tag="w", name=f"w{e}")
                nc.sync.dma_start(out=w_all, in_=wq[e])
                if e == 2:
                    # x/Wbar (phase-4 inputs) ride the sync ring behind the
                    # first three expert weights: the single queue keeps the
                    # full ~93 GB/s and its FIFO gives the head xq/w0
                    # transfers strict priority; these 4MB then transfer
                    # during the wpool slack and land ~90us before phase 4
                    nc.sync.dma_start(out=xT_sb, in_=xT[:, :, :])
                    nc.sync.dma_start(out=wbar_sb, in_=wbar[:, :, :])
                for oh in range(OH):
                    for nt in range(NT):
                        dcol = delta_at(nt)[:, e : e + 1]
                        acc = acc_sb[nt][oh]
                        # alternate psum between both pools -> ~8 banks in
                        # flight, drains never gate the PE
                        if (oh * NT + nt) % 2 == 0:
                            psum = mmp.tile([P, FO], F32, tag="mm")
                        else:
                            psum = gp.tile([P, FO], F32, tag="g", name="psum_e")
                        for dk in range(DK2):
                            nc.tensor.matmul(
                                psum,
                                xq_sb[:, dk, :, nt * P : (nt + 1) * P],
                                w_all[:, dk, :, oh * FO : (oh + 1) * FO],
                                start=(dk == 0),
                                stop=(dk == DK2 - 1),
                                perf_mode=DR,
                            )
                        if e == 0:
                            # first expert initializes acc; split ACT/DVE
                            if (oh * NT + nt) % 2 == 0:
                                nc.scalar.mul(acc, psum, dcol)
                            else:
                                nc.vector.tensor_scalar_mul(acc, psum, dcol)
                        else:
                            nc.vector.scalar_tensor_tensor(
                                out=acc,
                                in0=psum,
                                scalar=dcol,
                                in1=acc,
                                op0=mybir.AluOpType.mult,
                                op1=mybir.AluOpType.add,
                            )

            # ---- phase 4: bf16 Wbar pass + bias, fused final drains.
            # Per tile one 9-matmul PSUM group: 8 dk matmuls of x@Wbar plus
            # one bias matmul (gatesT[g] @ zero-padded per-strip b_e); the
            # drain adds acc and converts to bf16, then DMAs out ----
            for j, (nt, oh) in enumerate(
                [(nt, oh) for oh in range(OH) for nt in range(NT)]
            ):
                g, a = nt // 4, nt % 4
                pool, tg = ((mmp, "mm"), (gp, "g"))[j % 2]
                psum = pool.tile([P, FO], F32, tag=tg, name=f"wb{nt}_{oh}")
                for dk in range(DK):
                    nc.tensor.matmul(
                        psum,
                        xT_sb[:, dk, nt * P : (nt + 1) * P],
                        wbar_sb[:, dk, oh * FO : (oh + 1) * FO],
                        start=(dk == 0),
                        stop=False,
                    )
                nc.tensor.matmul(
                    psum,
                    gatesT[g],
                    be_sb[:, a, oh * FO : (oh + 1) * FO],
                    start=False,
                    stop=True,
                )
                if j == 2 * NT - 1:
                    # final tile: drain + DMA in 256-wide chunks so the DMA
                    # of the first half overlaps the drain of the second
                    for hh in range(2):
                        sl = slice(hh * 256, (hh + 1) * 256)
                        nc.vector.tensor_add(
                            out_sb[nt][oh][:, sl], acc_sb[nt][oh][:, sl],
                            psum[:, sl],
                        )
                        oslice = out[nt, oh, :, sl]
                        if hh == 0:
                            nc.sync.dma_start(out=oslice, in_=out_sb[nt][oh][:, sl])
                        else:
                            nc.scalar.dma_start(out=oslice, in_=out_sb[nt][oh][:, sl])
                else:
                    nc.vector.tensor_add(
                        out_sb[nt][oh], acc_sb[nt][oh], psum
                    )
                    oslice = out[nt, oh, :, :]
                    # outputs stay off the sync ring (which must drain the
                    # expert-weight stream) and off the scalar ring's
                    # x/Wbar transfers: alternate gpsimd/scalar
                    if j % 2 == 0:
                        nc.gpsimd.dma_start(out=oslice, in_=out_sb[nt][oh])
                    else:
                        nc.scalar.dma_start(out=oslice, in_=out_sb[nt][oh])

    legalize_single_wait(nc)
    return nc


_NC_CACHE = {}


def _get_nc():
    if "nc" not in _NC_CACHE:
        _NC_CACHE["nc"] = build_moe()
    return _NC_CACHE["nc"]


def make_in_maps(x, W_e, b_e, W_g1, b_g1, W_g2, b_g2):
    x = np.asarray(x, dtype=np.float32)
    W_e = np.asarray(W_e, dtype=np.float32)
    # fp8 expert weights, mean-centered (sum_e delta_e = 0 makes this
    # equivalent, and the smaller elements carry ~6% less fp8 noise):
    # [E, O, D] -> [E, P, DK2, 2, O] with logical d = dk2*256 + i*128 + p
    wbar_f = W_e.mean(axis=0)             # [O, D]
    wq = (
        ((W_e - wbar_f) * SW)
        .astype(F8)
        .transpose(0, 2, 1)               # [E, D, O]
        .reshape(E, DK2, 2, P, O)
        .transpose(0, 3, 1, 2, 4)         # [E, P, DK2, 2, O]
    )
    wq = np.ascontiguousarray(wq)
    # bf16 mean expert: [P, DK, O]
    wbar = wbar_f.T.reshape(DK, P, O).transpose(1, 0, 2)
    wbar = np.ascontiguousarray(wbar).astype(BF)
    # fp8 gating layer-1 weights, same d-pairing as xq
    wg1q = (
        (np.asarray(W_g1, dtype=np.float32) * SW)
        .astype(F8)
        .T.reshape(DK2, 2, P, H)
        .transpose(2, 0, 1, 3)            # [P, DK2, 2, H]
    )
    wg1q = np.ascontiguousarray(wg1q)
    # wg2 absorbs the gating dequant 1/(SX*SW); bg1 is pre-scaled by SX*SW
    wg2t = (
        (np.asarray(W_g2, dtype=np.float32) * DEQ)
        .T.reshape(H2, P, E)
        .transpose(1, 0, 2)
    )
    wg2t = np.ascontiguousarray(wg2t).astype(BF)
    bg1 = np.ascontiguousarray(
        (np.asarray(b_g1, dtype=np.float32) * (SX * SW)).reshape(H2, P).T
    )
    cexp_np = np.ascontiguousarray(
        np.broadcast_to(
            np.exp(np.asarray(b_g2, dtype=np.float32))[None, None, :], (P, 4, E)
        )
    ).astype(np.float32)
    be_c = np.ascontiguousarray(np.asarray(b_e, dtype=np.float32)).astype(BF)
    ident_np = np.eye(P, dtype=np.float32)
    xb = x.astype(BF)
    xq8 = (x * SX).astype(F8)
    in_maps = []
    for c in range(NCORES):
        xrow = xb[c * NLOC : (c + 1) * NLOC, :]
        xT_c = np.asarray(xrow.T).reshape(DK, P, NLOC).transpose(1, 0, 2)
        xT_c = np.ascontiguousarray(xT_c)
        xq_c = (
            np.asarray(xq8[c * NLOC : (c + 1) * NLOC, :].T)
            .reshape(DK2, 2, P, NLOC)
            .transpose(2, 0, 1, 3)        # [P, DK2, 2, NLOC]
        )
        xq_c = np.ascontiguousarray(xq_c)
        in_maps.append(
            {
                "xT": xT_c,
                "xq": xq_c,
                "wbar": wbar,
                "wq": wq,
                "wg1q": wg1q,
                "wg2t": wg2t,
                "bg1": bg1,
                "cexp": cexp_np,
                "be_c": be_c,
                "ident": ident_np,
            }
        )
    return in_maps


def kernel(x, W_e, b_e, W_g1, b_g1, W_g2, b_g2, **run_kwargs):
    nc = _get_nc()
    in_maps = make_in_maps(x, W_e, b_e, W_g1, b_g1, W_g2, b_g2)
    res = run_bass_kernel_spmd(nc, in_maps, core_ids=list(range(NCORES)), **run_kwargs)
    outs = []
    for c in range(NCORES):
        o = np.asarray(res.results[c]["out"])  # [NT, OH, P, FO] bf16
        outs.append(
            o.astype(np.float32).transpose(0, 2, 1, 3).reshape(NLOC, O)
        )
    out = np.concatenate(outs, axis=0)
    if run_kwargs:
        kernel.last_results = res
    return out


if __name__ == "__main__":
    rng = np.random.default_rng(0)
    s = 1.0 / np.sqrt(D)
    sh = 1.0 / np.sqrt(H)
    inputs = {
        "x": rng.standard_normal((N, D), dtype=np.float32),
        "W_e": rng.uniform(-s, s, (E, O, D)).astype(np.float32),
        "b_e": rng.uniform(-s, s, (E, O)).astype(np.float32),
        "W_g1": rng.uniform(-s, s, (H, D)).astype(np.float32),
        "b_g1": rng.uniform(-sh, sh, (H,)).astype(np.float32),
        "W_g2": rng.uniform(-sh, sh, (E, H)).astype(np.float32),
        "b_g2": rng.uniform(-sh, sh, (E,)).astype(np.float32),
    }
    out = kernel(**inputs)
    print("out", out.shape, out.dtype, float(np.abs(out).max()))
